# revision 32
# baseline (speedup 1.0000x reference)
"""Trainium2 Bass kernel for BertCounterFactLatentEditCompetitionTransformer.

Strategy:
  - Data-parallel over B (32 examples -> 4 per core x 8 cores).
  - Host folds weight algebra:
      * logits = (oq . ok) * scale collapses to q~ = Wq.T @ sfs + qb with
        Wq = oq_w.T @ ok_w, so the huge [B*K*L, D] x [D, D] "ok" projection
        is never computed.
      * repair_gain = (delta.T (M es + v2)) * scale with M = eq_w.T @ ek_w,
        eliminating the eq/ek projections.
  - x and weights are cast to bf16 on host; accumulation on device is f32.
  - Big matmuls are "vector stationary": small activation columns are the
    stationary operand, weight rows stream as the moving operand.
  - Per-example (b) pipelining: gate -> sfs -> q~ -> logits -> attn sums run
    per b as soon as that example's x tiles land, so the serial chain is
    hidden behind the DMA stream; the weight-heavy dense chain is batched
    across all 20 (b,k) rows at the end.
"""

import os
import numpy as np
import ml_dtypes

STAGE = os.environ.get("KERNEL_STAGE", "full")

B, K, L, D = 32, 5, 256, 1024
NCORES = 8
BP = B // NCORES  # 4 examples per core
R = BP * K        # 20 (b,k) rows per core
EDIT_MIN_W = 0.1
SCALE = 1.0 / 32.0  # 1/sqrt(D)

BF16 = ml_dtypes.bfloat16

_CACHE = {}


def _bcast_ap(bass, ap, parts):
    """Broadcast a DRAM row across `parts` partitions (0-stride partition dim)."""
    return bass.AP(tensor=ap.tensor, offset=ap.offset,
                   ap=[[0, parts]] + list(ap.ap))


def _rows_ap(bass, t, start, step, n):
    """Strided row view of a 2D DRAM AP: rows start, start+step, ..."""
    rstride = t.ap[0][0]
    width = t.ap[1]
    return bass.AP(tensor=t.tensor, offset=t.offset + start * rstride,
                   ap=[[step * rstride, n], list(width)])


def _build():
    import concourse.bass as bass
    import concourse.bacc as bacc
    import concourse.tile as tile
    import concourse.mybir as mybir
    from concourse.masks import make_identity
    from contextlib import ExitStack

    f32 = mybir.dt.float32
    bf16 = mybir.dt.bfloat16
    Alu = mybir.AluOpType
    Act = mybir.ActivationFunctionType
    X = mybir.AxisListType.X

    nc = bacc.Bacc("TRN2", target_bir_lowering=False, debug=False,
                   enable_asserts=False, num_devices=NCORES)

    def din(name, shape, dt):
        return nc.dram_tensor(name, shape, dt, kind="ExternalInput").ap()

    xd = din("x", [BP, 128, K, 2, D], bf16)   # host pre-transposed
    mcols_d = din("mcols", [128, BP * 11], f32)
    aw_d = din("aw", [D], bf16)
    ab_d = din("ab", [1], f32)
    qb_d = din("qb", [D], bf16)
    c0_d = din("c0", [1, 1], f32)
    db_d = din("db", [D], bf16)
    es1b_d = din("es1b", [D], bf16)
    f1b_d = din("f1b", [D], bf16)
    f2b_d = din("f2b", [D], bf16)
    es2b_d = din("es2b", [1], f32)
    minv2_d = din("minv2", [2, 1], f32)
    gam_d = din("gam", [D], bf16)
    bet_d = din("bet", [D], bf16)
    es2c_d = din("es2c", [128, 8], bf16)
    ur_d = din("ur", [1, D], bf16)
    v2r_d = din("v2r", [1, D], bf16)
    wq_d = din("wq", [D, D], bf16)
    wd_d = din("wd", [D, D], bf16)
    wm_d = din("wm", [D, D], bf16)
    wes1_d = din("wes1", [4 * D, D], bf16)
    wf1_d = din("wf1", [3 * D, D], bf16)
    wf2_d = din("wf2", [D, D], bf16)

    ev_d = nc.dram_tensor("ev", [BP, K, D], f32, kind="ExternalOutput").ap()
    sc_d = nc.dram_tensor("scores", [1, R], f32, kind="ExternalOutput").ap()

    with tile.TileContext(nc) as tc, ExitStack() as ctx:
        def psum_pool(name, bufs=1):
            cm = tc.tile_pool(name=name, bufs=bufs, space="PSUM")
            pool = cm.__enter__()
            pool._close_cm = cm
            return pool

        def close_pool(pool):
            pool._close_cm.__exit__(None, None, None)

        def sbuf_pool(name, bufs=1, side=None):
            kw = {} if side is None else {"side": side}
            cm = tc.tile_pool(name=name, bufs=bufs, **kw)
            pool = cm.__enter__()
            pool._close_cm = cm
            return pool

        consts = ctx.enter_context(tc.tile_pool(name="consts", bufs=1))
        work = ctx.enter_context(tc.tile_pool(name="work", bufs=1))
        scr = ctx.enter_context(tc.tile_pool(name="scr", bufs=3))
        ppt = psum_pool("ppt", bufs=2)
        ppc = psum_pool("ppc", bufs=1)

        # ---- constants (gpsimd SWDGE so SP/ACT sequencers stay free) ---
        ident = consts.tile([128, 128], bf16)
        make_identity(nc, ident)
        onescol = consts.tile([128, 1], bf16)
        nc.vector.memset(onescol, 1.0)
        ones1 = consts.tile([1, R], bf16)
        nc.vector.memset(ones1, 1.0)
        ones128 = consts.tile([1, 128], bf16)
        nc.vector.memset(ones128, 1.0)
        ones128f = consts.tile([1, 128], f32)
        nc.vector.memset(ones128f, 1.0)
        epsc = consts.tile([R, 1], f32)
        nc.vector.memset(epsc, 1e-5)

        # critical-path consts first (tiny)
        awb = consts.tile([128, D], bf16)
        nc.scalar.dma_start(out=awb, in_=_bcast_ap(bass, aw_d, 128))
        abb = consts.tile([128, 1], f32)
        nc.scalar.dma_start(out=abb, in_=_bcast_ap(bass, ab_d, 128))
        minv2 = consts.tile([2, 1], f32)
        nc.scalar.dma_start(out=minv2, in_=minv2_d)
        mct = consts.tile([128, BP * 11], f32)
        nc.scalar.dma_start(out=mct, in_=mcols_d)
        qbr = consts.tile([1, D], bf16)
        nc.scalar.dma_start(out=qbr, in_=_bcast_ap(bass, qb_d, 1))
        urow = consts.tile([1, D], bf16)
        nc.scalar.dma_start(out=urow, in_=ur_d)
        c0s = consts.tile([1, 1], f32)
        nc.scalar.dma_start(out=c0s, in_=c0_d)

        # x tiles: one DMA per example. Two right-side pools: {x2,x3} below
        # {x0,x1}; the top pool is released after example 1's sums finish,
        # freeing 40KB/partition for the stage-D weight-stream prefetch.
        xpB = sbuf_pool("xpB", side="left")
        xpA = sbuf_pool("xpA", side="right")
        wstr2 = sbuf_pool("wstr2", bufs=5, side="right")
        xb_t = [None] * BP
        for b in (2, 3, 0, 1):
            pool = xpA if b < 2 else xpB
            t = pool.tile([128, K, 2, D], bf16, name=f"xb{b}")
            xb_t[b] = t
        for b in range(BP):
            nc.sync.dma_start(out=xb_t[b], in_=xd[b])

        # resident q~ weights (ACT sequencer dispatch)
        wqs = consts.tile([128, 8, D], bf16)
        nc.scalar.dma_start(out=wqs,
                            in_=wq_d.rearrange("(c p) i -> p c i", p=128))

        es2bb = consts.tile([1, 1], f32)
        nc.scalar.dma_start(out=es2bb, in_=_bcast_ap(bass, es2b_d, 1))

        # DRAM bounce rows (for partition-relocating broadcasts)
        dpool = ctx.enter_context(tc.tile_pool(name="dpool", bufs=1,
                                               space="DRAM"))
        sfd = dpool.tile([2 * BP, D], bf16)

        # persistent work tiles
        sfall = work.tile([2 * BP, D], bf16)
        deltarows = work.tile([BP, D], bf16)
        featsrows = work.tile([R, 4 * D], bf16)
        evcat = work.tile([R, 3 * D], bf16)
        fscols = work.tile([128, 8, BP], bf16)
        deltacols = work.tile([128, 8, BP], bf16)
        featscols = work.tile([128, 32, R], bf16)
        evcatcols = work.tile([128, 24, R], bf16)
        h1cols = work.tile([128, 8, R], bf16)
        hfcols = work.tile([128, 8, R], bf16)
        zcols = work.tile([128, 8, R], bf16)
        h1rows = work.tile([R, D], bf16, tag="rowsh1")
        hfrows = work.tile([R, D], bf16, tag="rowsh1")
        zrows = work.tile([R, D], bf16, tag="rowz")
        tpre = work.tile([BP, D], f32, tag="bigf32", padded_shape=[R, D])
        evpre = work.tile([R, D], f32, tag="bigf32")
        rk20 = work.tile([R, 1], f32)
        eall = work.tile([128, 2, R], bf16)
        nc.vector.memset(eall, 0.0)

        _dot_i = [0]

        def dot_col(xs, vec, col):
            """col[p] = sum_d xs[p,d]*vec[p,d], load-balanced DVE/ACT."""
            i = _dot_i[0]
            _dot_i[0] += 1
            pr = scr.tile([128, D], bf16, tag="stt_l", bufs=3, name="pr")
            if i % 10 < 3:
                nc.vector.scalar_tensor_tensor(
                    out=pr, in0=xs, scalar=1.0, in1=vec,
                    op0=Alu.bypass, op1=Alu.mult, accum_out=col)
            else:
                nc.vector.tensor_mul(pr, xs, vec)
                nc.scalar.activation(pr, pr, Act.Copy, accum_out=col)

        def t_cols(rows, nrows, nchunk, dst):
            for c in range(nchunk):
                ptr = ppt.tile([128, nrows], bf16, tag="ptr", name="ptr")
                nc.tensor.transpose(
                    ptr, rows[0:nrows, c * 128:(c + 1) * 128],
                    ident[0:nrows, 0:nrows])
                nc.scalar.copy(dst[:, c, :], ptr)

        if STAGE == "dma":
            # just consume x so DMA runs; write dummy outputs
            nc.vector.memset(evpre, 0.0)
            for b in range(BP):
                nc.vector.tensor_copy(sfall[0:1, :], xb_t[b][:, 0, 0, :][0:1, :])
            nc.sync.dma_start(out=ev_d.rearrange("b k d -> (b k) d"),
                              in_=evpre)
            nc.sync.dma_start(out=sc_d, in_=evpre[0:1, 0:R])
            close_pool(ppc)
            close_pool(ppt)
            nc.compile()
            return nc

        # ====== per-example pipeline: gate/sfs -> q~ -> logits/es ======
        psE = [ppc.tile([R, 512], f32, tag=f"psE{dc}", name=f"psE{dc}")
               for dc in range(2)]
        qBs = {}
        cbbs = {}
        for b in range(BP):
            xt = xb_t[b]
            # ---- stage A_b: masked-softmax gate + sfs/sfc sums --------
            alc = scr.tile([128, K], f32, tag="alc")
            for k in range(K):
                dot_col(xt[:, k, 0, :], awb, alc[:, k:k + 1])
            alsum = scr.tile([128, 1], f32, tag="alsum")
            nc.vector.reduce_sum(alsum, alc, axis=X)
            egr = scr.tile([128, 1], f32, tag="egr")
            nc.scalar.activation(egr, alsum, Act.Exp, bias=abb, scale=1.0 / K)
            w01 = scr.tile([128, 2], bf16, tag="w01")
            fmc = mct[:, b * 11:b * 11 + 1]
            nc.vector.tensor_mul(w01[:, 0:1], egr, fmc)
            nc.vector.tensor_copy(w01[:, 1:2], fmc)
            psS = ppc.tile([2, 1], f32, tag="psS", name="psS")
            nc.tensor.matmul(psS, w01, onescol, start=True, stop=True,
                             skip_group_check=True)
            s2 = scr.tile([2, 1], f32, tag="s2")
            nc.scalar.copy(s2, psS)
            nc.vector.tensor_max(s2, s2, minv2)
            r2 = scr.tile([2, 1], f32, tag="r2")
            nc.vector.reciprocal(r2, s2)
            nc.vector.tensor_scalar_mul(r2, r2, 1.0 / K)
            sfb = scr.tile([2, D], bf16, tag="sfb", bufs=2)
            for dc in range(2):
                psA = ppc.tile([2, 512], f32, tag="psA", name="psA")
                for k in range(K):
                    nc.tensor.matmul(
                        psA, w01, xt[:, k, 0, dc * 512:(dc + 1) * 512],
                        start=(k == 0), stop=(k == K - 1),
                        skip_group_check=True)
                nc.vector.tensor_scalar(
                    out=sfb[:, dc * 512:(dc + 1) * 512], in0=psA,
                    scalar1=r2, scalar2=None, op0=Alu.mult)
            nc.sync.dma_start(out=_rows_ap(bass, sfd, b, BP, 2), in_=sfb)

            # ---- stage B_b: q~ = Wq.T sfs + qb; cb = u.sfs + c0 -------
            fsc = scr.tile([128, 8, 2], bf16, tag="fsc", bufs=2)
            t_cols(sfb, 2, 8, fsc)
            cbt = scr.tile([1, D], bf16, tag="cbt", bufs=1)
            cba = scr.tile([1, 1], f32, tag="cba")
            nc.vector.scalar_tensor_tensor(
                out=cbt, in0=sfb[0:1, :], scalar=1.0, in1=urow,
                op0=Alu.bypass, op1=Alu.mult, accum_out=cba)
            cbsb = scr.tile([1, 1], f32, tag="cbsb", bufs=2)
            nc.vector.tensor_scalar(out=cbsb, in0=cba, scalar1=c0s,
                                    scalar2=SCALE, op0=Alu.add, op1=Alu.mult)
            qrow = scr.tile([1, D], bf16, tag="qrow", bufs=2)
            for dc in range(2):
                psQ = ppc.tile([1, 512], f32, tag="psQ", name="psQ")
                for jc in range(8):
                    nc.tensor.matmul(
                        psQ, fsc[:, jc, 0:1],
                        wqs[:, jc, dc * 512:(dc + 1) * 512],
                        start=(jc == 0), stop=(jc == 7),
                        skip_group_check=True)
                nc.vector.tensor_add(qrow[:, dc * 512:(dc + 1) * 512], psQ,
                                     qbr[:, dc * 512:(dc + 1) * 512])
            qB = scr.tile([128, D], bf16, tag="qB", bufs=4)
            for dc in range(2):
                psqb = ppc.tile([128, 512], f32, tag="psQ", name="psqb")
                nc.tensor.matmul(psqb, ones128,
                                 qrow[:, dc * 512:(dc + 1) * 512],
                                 start=True, stop=True, skip_group_check=True)
                nc.scalar.copy(qB[:, dc * 512:(dc + 1) * 512], psqb)
            cbb = scr.tile([128, 1], f32, tag="cbb", bufs=4)
            pscb = ppc.tile([128, 1], f32, tag="psS", name="pscb")
            nc.tensor.matmul(pscb, ones128f, cbsb, start=True, stop=True,
                             skip_group_check=True)
            nc.scalar.copy(cbb, pscb)
            qBs[b] = qB
            cbbs[b] = cbb

        wd_tiles = []
        es1_tiles = []
        wm_tiles = []
        for dram, nchunks, tiles in ((wd_d, 2, wd_tiles),
                                     (wes1_d, 8, es1_tiles),
                                     (wm_d, 2, wm_tiles)):
            for c4 in range(nchunks):
                wt = wstr2.tile([128, 4, D], bf16, tag="wst2", name="wst2")
                nc.scalar.dma_start(
                    out=wt, in_=dram[c4 * 512:(c4 + 1) * 512, :].rearrange(
                        "(c p) i -> p c i", p=128))
                tiles.append(wt)

        for b in range(BP):
            xt = xb_t[b]
            qB = qBs[b]
            cbb = cbbs[b]
            # ---- stage C_b: logits -> exp -> masked weighted sums -----
            scols = scr.tile([128, 2, K], f32, tag="scols")
            for lh in range(2):
                for k in range(K):
                    dot_col(xt[:, k, lh, :], qB, scols[:, lh, k:k + 1])
            etmp = scr.tile([128, 2, K], f32, tag="etmp")
            nc.scalar.activation(etmp, scols, Act.Exp, bias=cbb, scale=SCALE)
            omv = mct[:, b * 11 + 1:b * 11 + 11].rearrange(
                "p (j k) -> p j k", j=2)
            nc.vector.tensor_mul(eall[:, :, b * K:(b + 1) * K], etmp, omv)
            for k in range(K):
                bk = b * K + k
                ebk = scr.tile([128, 2, R], bf16, tag="ebk")
                nc.vector.memset(ebk, 0.0)
                nc.vector.tensor_copy(ebk[:, :, bk:bk + 1],
                                      eall[:, :, bk:bk + 1])
                for lh in range(2):
                    for dc in range(2):
                        nc.tensor.matmul(
                            psE[dc], ebk[:, lh, :],
                            xt[:, k, lh, dc * 512:(dc + 1) * 512],
                            start=(b == 0 and k == 0 and lh == 0),
                            stop=(b == BP - 1 and k == K - 1 and lh == 1),
                            skip_group_check=True)
        close_pool(xpB)
        biasp = sbuf_pool("biasp", side="left")
        dbrows = biasp.tile([BP, D], bf16)
        nc.scalar.dma_start(out=dbrows, in_=_bcast_ap(bass, db_d, BP))
        es1brows = biasp.tile([R, D], bf16)
        nc.scalar.dma_start(out=es1brows, in_=_bcast_ap(bass, es1b_d, R))
        f1brows = biasp.tile([R, D], bf16)
        nc.scalar.dma_start(out=f1brows, in_=_bcast_ap(bass, f1b_d, R))
        f2brows = biasp.tile([R, D], bf16)
        nc.scalar.dma_start(out=f2brows, in_=_bcast_ap(bass, f2b_d, R))
        gamrows = biasp.tile([R, D], bf16)
        nc.scalar.dma_start(out=gamrows, in_=_bcast_ap(bass, gam_d, R))
        betrows = biasp.tile([R, D], bf16)
        nc.scalar.dma_start(out=betrows, in_=_bcast_ap(bass, bet_d, R))
        es2cols = biasp.tile([128, 8], bf16)
        nc.scalar.dma_start(out=es2cols, in_=es2c_d)
        v2s = biasp.tile([1, D], bf16)
        nc.scalar.dma_start(out=v2s, in_=v2r_d)
        wstr3 = sbuf_pool("wstr3", bufs=3, side="left")
        f1_tiles = []
        f2_tiles = []
        for dram, nchunks, tiles in ((wf1_d, 6, f1_tiles),
                                     (wf2_d, 2, f2_tiles)):
            for c4 in range(nchunks):
                wt = wstr3.tile([128, 4, D], bf16, tag="wst3", name="wst3")
                nc.scalar.dma_start(
                    out=wt, in_=dram[c4 * 512:(c4 + 1) * 512, :].rearrange(
                        "(c p) i -> p c i", p=128))
                tiles.append(wt)

        # ====== attention normalizers + es rows ========================
        psZ = ppc.tile([R, 1], f32, tag="psZ", name="psZ")
        for lh in range(2):
            nc.tensor.matmul(psZ, eall[:, lh, :], onescol,
                             start=(lh == 0), stop=(lh == 1),
                             skip_group_check=True)
        zs = work.tile([R, 1], f32)
        nc.scalar.copy(zs, psZ)
        nc.vector.tensor_scalar_max(zs, zs, 1e-8)
        nc.vector.reciprocal(rk20, zs)
        for dc in range(2):
            nc.vector.tensor_scalar(
                out=featsrows[:, D + dc * 512:D + (dc + 1) * 512],
                in0=psE[dc], scalar1=rk20, scalar2=None, op0=Alu.mult)
        nc.sync.dma_start(out=sfall, in_=sfd)
        close_pool(ppc)
        if STAGE == "abc":
            nc.vector.memset(evpre, 0.0)
            nc.sync.dma_start(out=ev_d.rearrange("b k d -> (b k) d"),
                              in_=evpre)
            nc.sync.dma_start(out=sc_d, in_=evpre[0:1, 0:R])
            close_pool(ppt)
            nc.compile()
            return nc

        # ====== stage D: batched dense chain ===========================
        ppD0 = psum_pool("ppD0")
        t_cols(sfall, BP, 8, fscols)
        for b in range(BP):
            nc.scalar.dma_start(out=featsrows[b * K:(b + 1) * K, 0:D],
                                in_=_bcast_ap(bass, sfd[BP + b], K))
            nc.scalar.dma_start(out=evcat[b * K:(b + 1) * K, 0:D],
                                in_=_bcast_ap(bass, sfd[b], K))
        # delta = tanh(d_w sfs + d_b)
        psD = [ppD0.tile([BP, 512], f32, tag=f"psD{dc}", name=f"psD{dc}")
               for dc in range(2)]
        for c4 in range(2):
            wt = wd_tiles[c4]
            for c in range(4):
                jc = c4 * 4 + c
                for dc in range(2):
                    nc.tensor.matmul(psD[dc], fscols[:, jc, :],
                                     wt[:, c, dc * 512:(dc + 1) * 512],
                                     start=(jc == 0), stop=(jc == 7))
        for dc in range(2):
            nc.vector.tensor_add(tpre[:, dc * 512:(dc + 1) * 512], psD[dc],
                                 dbrows[:, dc * 512:(dc + 1) * 512])
        nc.scalar.activation(deltarows, tpre, Act.Tanh)
        t_cols(deltarows, BP, 8, deltacols)
        # feats = [fc | es | |fc-es| | fc*es];  evcat = [fs | es | fs*es]
        dtmp = work.tile([R, D], bf16, tag="rowz")
        nc.vector.tensor_sub(dtmp, featsrows[:, 0:D], featsrows[:, D:2 * D])
        nc.scalar.activation(featsrows[:, 2 * D:3 * D], dtmp, Act.Abs)
        nc.vector.tensor_mul(featsrows[:, 3 * D:4 * D],
                             featsrows[:, 0:D], featsrows[:, D:2 * D])
        nc.vector.tensor_copy(evcat[:, D:2 * D], featsrows[:, D:2 * D])
        nc.vector.tensor_mul(evcat[:, 2 * D:3 * D],
                             evcat[:, 0:D], featsrows[:, D:2 * D])
        t_cols(featsrows, R, 32, featscols)
        close_pool(ppD0)
        # es1
        ppD1 = psum_pool("ppD1")
        psH = [ppD1.tile([R, 512], f32, tag=f"psH{dc}", name=f"psH{dc}")
               for dc in range(2)]
        for c4 in range(8):
            wt = es1_tiles[c4]
            for c in range(4):
                jc = c4 * 4 + c
                for dc in range(2):
                    nc.tensor.matmul(psH[dc], featscols[:, jc, :],
                                     wt[:, c, dc * 512:(dc + 1) * 512],
                                     start=(jc == 0), stop=(jc == 31))
        for dc in range(2):
            hadd = scr.tile([R, 512], f32, tag="psadd", bufs=2)
            nc.vector.tensor_add(hadd, psH[dc],
                                 es1brows[:, dc * 512:(dc + 1) * 512])
            nc.vector.tensor_scalar_max(
                h1rows[:, dc * 512:(dc + 1) * 512], hadd, 0.0)
        t_cols(h1rows, R, 8, h1cols)
        close_pool(ppD1)
        # eg / z / rg / mp
        ppD2 = psum_pool("ppD2")
        psEG = ppD2.tile([1, R], f32, tag="psEG")
        for oc in range(8):
            nc.tensor.matmul(psEG, es2cols[:, oc:oc + 1], h1cols[:, oc, :],
                             start=(oc == 0), stop=(oc == 7))
        egrow = work.tile([1, R], f32)
        nc.scalar.copy(egrow, psEG)
        psZm = [ppD2.tile([R, 512], f32, tag=f"psZm{dc}", name=f"psZm{dc}")
                for dc in range(2)]
        for c4 in range(2):
            wt = wm_tiles[c4]
            for c in range(4):
                jc = c4 * 4 + c
                for dc in range(2):
                    nc.tensor.matmul(psZm[dc], featscols[:, 8 + jc, :],
                                     wt[:, c, dc * 512:(dc + 1) * 512],
                                     start=(jc == 0), stop=False,
                                     skip_group_check=True)
        for dc in range(2):
            nc.tensor.matmul(psZm[dc], ones1, v2s[:, dc * 512:(dc + 1) * 512],
                             start=False, stop=True, skip_group_check=True)
            nc.scalar.copy(zrows[:, dc * 512:(dc + 1) * 512], psZm[dc])
        t_cols(zrows, R, 8, zcols)
        prod = work.tile([128, R, 8], bf16)
        for k in range(K):
            nc.vector.tensor_mul(
                prod[:, k::K, :],
                zcols[:, :, k::K].rearrange("p o b -> p b o"),
                deltacols.rearrange("p o b -> p b o"))
        psRG = ppD2.tile([1, R * 8], f32, tag="psRG")
        nc.tensor.matmul(psRG, onescol, prod.rearrange("p a o -> p (a o)"),
                         start=True, stop=True)
        rgrow = work.tile([1, R], f32)
        nc.vector.reduce_sum(rgrow, psRG.rearrange("p (a o) -> p a o", o=8),
                             axis=X)
        sq = work.tile([128, BP, 8], bf16)
        nc.vector.tensor_mul(sq, deltacols.rearrange("p o b -> p b o"),
                             deltacols.rearrange("p o b -> p b o"))
        psMP = ppD2.tile([1, BP * 8], f32, tag="psMP")
        nc.tensor.matmul(psMP, onescol, sq.rearrange("p a o -> p (a o)"),
                         start=True, stop=True)
        mprow = work.tile([1, BP], f32)
        nc.vector.reduce_sum(mprow, psMP.rearrange("p (a o) -> p a o", o=8),
                             axis=X)
        t_cols(evcat, R, 24, evcatcols)
        close_pool(ppD2)
        # f1 / f2
        ppD3 = psum_pool("ppD3")
        psF = [ppD3.tile([R, 512], f32, tag=f"psF{dc}", name=f"psF{dc}")
               for dc in range(2)]
        for c4 in range(6):
            wt = f1_tiles[c4]
            for c in range(4):
                jc = c4 * 4 + c
                for dc in range(2):
                    nc.tensor.matmul(psF[dc], evcatcols[:, jc, :],
                                     wt[:, c, dc * 512:(dc + 1) * 512],
                                     start=(jc == 0), stop=(jc == 23))
        for dc in range(2):
            fadd = scr.tile([R, 512], f32, tag="psadd", bufs=2)
            nc.vector.tensor_add(fadd, psF[dc],
                                 f1brows[:, dc * 512:(dc + 1) * 512])
            nc.vector.tensor_scalar_max(
                hfrows[:, dc * 512:(dc + 1) * 512], fadd, 0.0)
        t_cols(hfrows, R, 8, hfcols)
        psEV = [ppD3.tile([R, 512], f32, tag=f"psEV{dc}", name=f"psEV{dc}")
                for dc in range(2)]
        for c4 in range(2):
            wt = f2_tiles[c4]
            for c in range(4):
                jc = c4 * 4 + c
                for dc in range(2):
                    nc.tensor.matmul(psEV[dc], hfcols[:, jc, :],
                                     wt[:, c, dc * 512:(dc + 1) * 512],
                                     start=(jc == 0), stop=(jc == 7))
        for dc in range(2):
            nc.vector.tensor_add(evpre[:, dc * 512:(dc + 1) * 512], psEV[dc],
                                 f2brows[:, dc * 512:(dc + 1) * 512])
        # layernorm
        stats = work.tile([R, 2, 6], f32)
        for i in range(2):
            nc.vector.bn_stats(stats[:, i, :], evpre[:, i * 512:(i + 1) * 512])
        mv = work.tile([R, 2], f32)
        nc.vector.bn_aggr(mv, stats)
        sd = work.tile([R, 1], f32)
        nc.scalar.activation(sd, mv[:, 1:2], Act.Sqrt, bias=epsc, scale=1.0)
        rstd = work.tile([R, 1], f32)
        nc.vector.reciprocal(rstd, sd)
        nc.vector.tensor_scalar(out=evpre, in0=evpre, scalar1=mv[:, 0:1],
                                scalar2=rstd, op0=Alu.subtract, op1=Alu.mult)
        nc.vector.tensor_mul(evpre, evpre, gamrows)
        nc.vector.tensor_add(evpre, evpre, betrows)
        nc.sync.dma_start(out=ev_d.rearrange("b k d -> (b k) d"), in_=evpre)
        # scores
        srow = work.tile([1, R], f32)
        nc.vector.tensor_scalar(out=srow, in0=egrow, scalar1=es2bb,
                                scalar2=None, op0=Alu.add)
        rgs = work.tile([1, R], f32)
        nc.vector.tensor_scalar_mul(rgs, rgrow, 0.15 * SCALE)
        nc.vector.tensor_add(srow, srow, rgs)
        mps = work.tile([1, BP], f32)
        nc.vector.tensor_scalar_mul(mps, mprow, -0.05 * EDIT_MIN_W)
        srowv = srow.rearrange("p (b k) -> p b k", k=K)
        for kk in range(K):
            nc.vector.tensor_add(srowv[:, :, kk], srowv[:, :, kk], mps)
        mrow = work.tile([1, BP], f32)
        nc.vector.reduce_sum(mrow, srowv, axis=X)
        nc.vector.tensor_scalar_mul(mrow, mrow, 1.0 / K)
        for kk in range(K):
            nc.vector.tensor_sub(srowv[:, :, kk], srowv[:, :, kk], mrow)
        nc.sync.dma_start(out=sc_d, in_=srow)
        close_pool(wstr3)
        close_pool(biasp)
        close_pool(wstr2)
        close_pool(xpA)
        close_pool(ppD3)
        close_pool(ppt)

    nc.compile()
    return nc


def _host_prep(inputs):
    f32 = np.float32
    x = np.asarray(inputs["x"], f32)
    ids = np.asarray(inputs["x_ids"])
    pad_idx = int(np.asarray(inputs["pad_idx"]))
    sep_idx = int(np.asarray(inputs["sep_idx"]))

    idsf = ids.reshape(B * K, L)
    valid = idsf != pad_idx
    sepm = idsf == sep_idx
    has_sep = sepm.any(1)
    idxs = sepm.argmax(1)
    vlen = valid.sum(1)
    fallback = np.clip(vlen // 2, 1, max(1, L - 2))
    sep_pos = np.where(has_sep, idxs, fallback)
    pos = np.arange(L)[None, :]
    sp = sep_pos[:, None]
    eos = np.clip(vlen[:, None] - 1, 0, None)
    fm = ((pos > 0) & (pos < sp) & valid).reshape(B, K, L)
    om = ((pos > sp) & (pos < eos) & valid).reshape(B, K, L)
    shared_fm = fm[:, 0, :].astype(f32)
    om = om.astype(f32)

    mcols = np.zeros((B, 11, 128), f32)
    mcols[:, 0, :] = shared_fm[:, :128]
    om_r = om.reshape(B, K, 2, 128)
    for lh in range(2):
        for k in range(K):
            mcols[:, 1 + lh * K + k, :] = om_r[:, k, lh, :]
    m_core = mcols.reshape(NCORES, BP * 11, 128).transpose(0, 2, 1).copy()

    g = lambda n: np.asarray(inputs[n], f32)
    oq_w, oq_b = g("oq_w"), g("oq_b")
    ok_w, ok_b = g("ok_w"), g("ok_b")
    d_w, d_b = g("d_w"), g("d_b")
    eq_w = g("eq_w")
    ek_w, ek_b = g("ek_w"), g("ek_b")
    es1_w, es1_b = g("es1_w"), g("es1_b")
    es2_w, es2_b = g("es2_w"), g("es2_b")
    f1_w, f1_b = g("f1_w"), g("f1_b")
    f2_w, f2_b = g("f2_w"), g("f2_b")

    com = {
        "aw": g("a_w").reshape(D).astype(BF16),
        "ab": g("a_b").reshape(1),
        "qb": (ok_w.T @ oq_b).astype(BF16),
        "c0": np.array([[oq_b @ ok_b]], f32),
        "db": d_b.astype(BF16),
        "es1b": es1_b.astype(BF16),
        "f1b": f1_b.astype(BF16),
        "f2b": f2_b.astype(BF16),
        "es2b": es2_b.reshape(1),
        "minv2": np.array([[1e-8], [1.0]], f32),
        "gam": g("ln_g").astype(BF16),
        "bet": g("ln_b").astype(BF16),
        "es2c": es2_w.reshape(8, 128).T.copy().astype(BF16),
        "ur": (oq_w.T @ ok_b).reshape(1, D).astype(BF16),
        "v2r": (eq_w.T @ ek_b).reshape(1, D).astype(BF16),
        "wq": (oq_w.T @ ok_w).astype(BF16),
        "wd": d_w.T.copy().astype(BF16),
        "wm": (ek_w.T @ eq_w).astype(BF16),
        "wes1": es1_w.T.copy().astype(BF16),
        "wf1": f1_w.T.copy().astype(BF16),
        "wf2": f2_w.T.copy().astype(BF16),
    }
    # x pre-transposed to [BP, 128(p), K, 2(lh), D] per core
    xb = x.astype(BF16).reshape(NCORES, BP, K, 2, 128, D)
    xb = np.ascontiguousarray(xb.transpose(0, 1, 4, 2, 3, 5))
    in_maps = []
    for c in range(NCORES):
        m = dict(com)
        m["x"] = xb[c]
        m["mcols"] = np.ascontiguousarray(m_core[c])
        in_maps.append(m)
    return in_maps


def kernel(**inputs):
    from concourse import bass_utils
    if "nc" not in _CACHE:
        _CACHE["nc"] = _build()
    nc = _CACHE["nc"]
    in_maps = _host_prep(inputs)
    res = bass_utils.run_bass_kernel_spmd(nc, in_maps,
                                          core_ids=list(range(NCORES)))
    ev = np.concatenate([res.results[c]["ev"] for c in range(NCORES)], axis=0)
    sc = np.concatenate(
        [res.results[c]["scores"].reshape(BP, K) for c in range(NCORES)],
        axis=0)
    return ev.astype(np.float32), sc.astype(np.float32)
